# revision 20
# baseline (speedup 1.0000x reference)
"""Trainium2 Bass kernel for nn_Classifier_42588895707508.

Computation (see reference):
    pool_k[b, h] = max_{s < eff_k[b]} x_k[b, s, h]      (k = 1, 2)
    out[b, c]    = sum_h pool_1[b,h] W[c,h] + pool_2[b,h] W[c, 768+h] + bias[c]
where eff_k[b] is derived from the mask m_k (index of first zero; 0 -> S).

Strategy (memory-bound, ragged sequences):
  * Only the valid prefix of each sample touches the device (~50% of the
    input), packed as fp16 (2e-2 output tolerance; fp16 pooling+matmul
    lands at ~3e-4), halving HBM traffic vs fp32.
  * Host packs, per core, transposed valid blocks x_k[b, :eff, :].T
    (h on partitions, s on the free dim). Rows (= (kind, sample) pairs)
    are distributed round-robin by length rank across the 8 cores, so
    every core gets an identical width structure -> one SPMD program,
    balanced load.
  * DVE reduce_max (InstTensorReduce) has NO accelerated perf modes
    (1 elem/lane/cycle), but tensor_tensor max supports 2x_1p (2 elem/
    lane/cycle for 16-bit). So each DMA tile is laid out as F=8 "planes":
    seq positions of every slot are split across the planes, and the
    planes are folded pairwise with 7 full-tile tensor_max ops (2x rate)
    into a ping-pong accumulator; only the remaining 1/8 of the columns
    go through the slow tensor_reduce. DVE cost: ~0.56 cycles/elem vs
    1.0, dropping it safely below the DMA roofline.
  * Slots are grouped into equal-width reduce groups (widths rounded to
    multiples of F) so one reduce instruction handles g slots x 6 chunks.
    Narrow many-instruction groups are streamed first (overhead hides
    under the DMA stream); wide groups last, with small final tiles so
    the post-DMA tail is short.
  * The tiny linear layer runs on the tensor engine (fp16 matmul, K=128
    per chunk accumulated in fp32 PSUM); per-row partials are summed on
    the host (x1/x2 rows of one sample may land on different cores).
"""

import numpy as np

B, S, H, C = 512, 256, 768, 2
NCORES = 8
CH = H // 128  # 6 h-chunks of 128 partitions
KINDS = 2
SLOTS = B // NCORES  # 64 slots per kind per core
NEG = np.float16(-65504.0)

F = 8  # fold planes per tile
PAD_COST = 1.0  # DP cost of one padded slot-column
INSTR_COST = 20.0  # DP cost of one extra reduce instruction
GROUP_CAP = 6144  # max real columns of one group
TILE_CAP = 28672  # max real columns of one DMA tile
RAMP_UP = [4096, 12288]
TAIL_CAP = 4096  # tile cap near the end of the stream (short DVE tail)
TAIL_ZONE = 8192  # switch to TAIL_CAP when this many columns remain
DATA_BUFS = 3
ACCW_MAX = TILE_CAP // F


def _eff_lengths(m):
    am = np.argmin(np.asarray(m), axis=1)
    return np.where(am == 0, S, am).astype(np.int64)


def _plan_groups(widths):
    """Partition the (descending) width list into contiguous groups.

    Group width is the max member width rounded up to a multiple of F.
    Returns list of (start, n, gw) minimizing PAD_COST * padding +
    INSTR_COST per group, via O(n^2) DP. A group must fit in GROUP_CAP.
    """
    n = len(widths)
    best = np.full(n + 1, np.inf)
    best[0] = 0.0
    prev = np.zeros(n + 1, dtype=np.int64)
    for i in range(1, n + 1):
        for j in range(i - 1, -1, -1):
            w = -(-int(widths[j]) // F) * F  # ceil to multiple of F
            if (i - j) * 6 * w > GROUP_CAP:
                break
            pad = PAD_COST * ((i - j) * w - widths[j:i].sum())
            cost = best[j] + pad + INSTR_COST
            if cost < best[i]:
                best[i] = cost
                prev[i] = j
    groups = []
    i = n
    while i > 0:
        j = prev[i]
        groups.append((j, i - j, -(-int(widths[j]) // F) * F))
        i = j
    groups.reverse()
    return groups


def _build_program(tiles):
    """Build the SPMD Bass program.

    tiles: list of (c0, acc_w, [(o, out_slot, n, v), ...]) in real DRAM
    columns; each tile spans 8*acc_w columns = 8 planes of acc_w; group
    plane-offsets o are tile-local.
    """
    import concourse.bacc as bacc
    import concourse.mybir as mybir
    from concourse.tile import TileContext

    R = sum(F * acc_w for _, acc_w, _ in tiles)
    nc = bacc.Bacc("TRN2", target_bir_lowering=False, debug=False, num_devices=NCORES)
    p_in = nc.dram_tensor("p", [128, R], mybir.dt.float16, kind="ExternalInput")
    wt_in = nc.dram_tensor(
        "wt", [128, KINDS * CH, C], mybir.dt.float16, kind="ExternalInput"
    )
    out_d = nc.dram_tensor(
        "out", [C, 2 * SLOTS], mybir.dt.float32, kind="ExternalOutput"
    )

    with TileContext(nc) as tc:
        with (
            tc.tile_pool(name="data", bufs=DATA_BUFS) as data_pool,
            tc.tile_pool(name="work", bufs=1) as work_pool,
            tc.tile_pool(name="psum", bufs=1, space="PSUM") as psum_pool,
        ):
            wt_t = work_pool.tile([128, KINDS * CH, C], mybir.dt.float16, tag="wt")
            acc = work_pool.tile([128, 2, ACCW_MAX], mybir.dt.float16, tag="acc")
            # pooled[p, slot, ch]: slot = kind*64 + i, partition p = h in chunk
            pooled = work_pool.tile(
                [128, KINDS * SLOTS, CH], mybir.dt.float16, tag="pooled"
            )

            first = True
            for c0, acc_w, tgroups in tiles:
                tw = F * acc_w
                dt = data_pool.tile([128, TILE_CAP], mybir.dt.float16, tag="data")
                # plane-pair sub-DMAs: fold j can start as soon as its pair
                # lands instead of waiting for the whole tile
                for s in range(0, F, 2):
                    lo, hi = s * acc_w, (s + 2) * acc_w
                    nc.sync.dma_start(
                        out=dt[:, lo:hi], in_=p_in[:, c0 + lo : c0 + hi]
                    )
                    if first:
                        # tiny weight load on the ACT HWDGE queue so the
                        # sync queue carries nothing but the data stream
                        nc.scalar.dma_start(out=wt_t, in_=wt_in[:, :, :])
                        first = False
                prev = dt[:, 0:acc_w]
                for j in range(1, F):
                    cur = acc[:, j % 2, :acc_w]
                    nc.vector.tensor_max(
                        out=cur, in0=prev, in1=dt[:, j * acc_w : (j + 1) * acc_w]
                    )
                    prev = cur
                for o, out_slot, gn, v in tgroups:
                    view = prev[:, o : o + gn * 6 * v].rearrange(
                        "p (g v) -> p g v", v=v
                    )
                    nc.vector.reduce_max(
                        out=pooled[:, out_slot : out_slot + gn, :],
                        in_=view,
                        axis=mybir.AxisListType.X,
                    )

            out_sb = work_pool.tile([C, 2 * SLOTS], mybir.dt.float32, tag="osb")
            for k in range(KINDS):
                ps = psum_pool.tile([C, SLOTS], mybir.dt.float32, tag=f"ps{k}")
                for ch in range(CH):
                    nc.tensor.matmul(
                        ps,
                        lhsT=wt_t[:, k * CH + ch, :],
                        rhs=pooled[:, k * SLOTS : (k + 1) * SLOTS, ch],
                        start=(ch == 0),
                        stop=(ch == CH - 1),
                    )
                nc.scalar.copy(out=out_sb[:, k * SLOTS : (k + 1) * SLOTS], in_=ps)
            nc.scalar.dma_start(out=out_d[:, :], in_=out_sb)

    nc.compile()
    return nc


def _plan(effs):
    """Shared planning: groups -> emission order -> tiles -> slot map.

    Returns (tiles, slotinfo, R) where slotinfo[k][i] =
    (c0, acc_w, po, v, w) for packing and tiles drive the program.
    """
    orders = [np.argsort(-effs[k], kind="stable") for k in range(KINDS)]
    slot_w = [effs[k][orders[k][:: NCORES]].astype(np.int64) for k in range(KINDS)]

    raw_groups = []  # (kind, start, n, w)
    for k in range(KINDS):
        for start, n, gw in _plan_groups(slot_w[k]):
            raw_groups.append((k, start, n, gw))
    # narrowest groups first: many small reduces hide under the DMA
    # stream; wide efficient groups last keep the post-DMA tail short
    raw_groups.sort(key=lambda g: g[3])

    total = sum(n * 6 * w for _, _, n, w in raw_groups)

    # assemble tiles of whole groups
    tiles = []  # (c0, acc_w, [(o, out_slot, n, v)])
    slotinfo = [[None] * SLOTS for _ in range(KINDS)]
    col = 0  # real columns consumed (tile bases)
    cur = None  # [c0, pw_sum, [(o, out_slot, n, v)], members]
    consumed = 0

    def close(cur):
        tiles.append((cur[0], cur[1], cur[2]))

    for k, start, n, w in raw_groups:
        v = w // F
        pw = n * 6 * v
        rw = F * pw
        remaining = total - consumed
        cap = (
            RAMP_UP[len(tiles)]
            if len(tiles) < len(RAMP_UP)
            else (TAIL_CAP if remaining <= TAIL_ZONE else TILE_CAP)
        )
        if cur is not None and cur[1] > 0 and F * cur[1] + rw > cap:
            # adding this group would exceed the cap -> close tile
            # (a lone oversized group still gets its own tile)
            close(cur)
            col += F * cur[1]
            cur = None
        if cur is None:
            cur = [col, 0, []]
        o = cur[1]
        cur[2].append((o, k * SLOTS + start, n, v))
        for j in range(n):
            slotinfo[k][start + j] = (cur[0], None, o + j * 6 * v, v, w)
        cur[1] += pw
        consumed += rw
    if cur is not None:
        close(cur)
        col += F * cur[1]
    R = col

    # fix up acc_w in slotinfo (known only at tile close)
    accw_by_c0 = {c0: acc_w for c0, acc_w, _ in tiles}
    for k in range(KINDS):
        for i in range(SLOTS):
            c0, _, po, v, w = slotinfo[k][i]
            slotinfo[k][i] = (c0, accw_by_c0[c0], po, v, w)

    assert all(F * acc_w <= TILE_CAP for _, acc_w, _ in tiles)
    assert R == total
    return tiles, slotinfo, orders, R


_NC_CACHE = {}


def kernel(x1, x2, m1, m2, W, b, _run_opts=None):
    from concourse.bass_utils import run_bass_kernel_spmd

    x1 = np.asarray(x1)
    x2 = np.asarray(x2)
    W = np.asarray(W, dtype=np.float32)
    b = np.asarray(b, dtype=np.float32)
    effs = [_eff_lengths(m1), _eff_lengths(m2)]
    tiles, slotinfo, orders, R = _plan(effs)

    # pack per-core data in fp16, fold-plane layout
    x16 = [np.asarray(x, dtype=np.float32).astype(np.float16) for x in (x1, x2)]
    packs = np.full((NCORES, 128, R), NEG, dtype=np.float16)
    for k in range(KINDS):
        xk, eff, order = x16[k], effs[k], orders[k]
        for i in range(SLOTS):
            c0, acc_w, po, v, w = slotinfo[k][i]
            for c in range(NCORES):
                bidx = order[i * NCORES + c]
                e = int(eff[bidx])
                dst = (
                    packs[c][:, c0 : c0 + F * acc_w]
                    .reshape(128, F, acc_w)[:, :, po : po + 6 * v]
                    .reshape(128, F, 6, v)
                )
                src = xk[bidx, :e, :].T.reshape(6, 128, e).transpose(1, 0, 2)
                jf, rem = e // v, e % v
                if jf:
                    dst[:, :jf] = (
                        src[:, :, : jf * v]
                        .reshape(128, 6, jf, v)
                        .transpose(0, 2, 1, 3)
                    )
                if rem:
                    dst[:, jf, :, :rem] = src[:, :, jf * v :]

    # weights, laid out so lhsT slices are [128 (h), C] per (kind, chunk)
    wtp = (
        np.ascontiguousarray(W.reshape(C, KINDS, CH, 128).transpose(3, 1, 2, 0))
        .reshape(128, KINDS * CH, C)
        .astype(np.float16)
    )

    key = tuple((c0, acc_w, tuple(tg)) for c0, acc_w, tg in tiles)
    nc = _NC_CACHE.get(key)
    if nc is None:
        nc = _build_program(tiles)
        _NC_CACHE[key] = nc
    in_maps = [{"p": packs[c], "wt": wtp} for c in range(NCORES)]

    res = None
    last_err = None
    for _attempt in range(3):
        try:
            res = run_bass_kernel_spmd(
                nc, in_maps, core_ids=list(range(NCORES)), **(_run_opts or {})
            )
            break
        except Exception as e:  # wedged device etc. -- retry
            last_err = e
    if res is None:
        raise last_err

    # combine per-row partials
    out_full = np.zeros((B, C), dtype=np.float32)
    res_all = np.stack([res.results[c]["out"] for c in range(NCORES)])  # [8, C, 128]
    for k in range(KINDS):
        part = res_all[:, :, k * SLOTS : (k + 1) * SLOTS]  # [core, C, slot]
        part = part.transpose(2, 0, 1).reshape(B, C)  # [(slot, core), C]
        out_full[orders[k]] += part
    out_full += b[None, :]
    if _run_opts is not None:
        kernel._last_res = res
    return out_full


# revision 21
# speedup vs baseline: 1.1488x; 1.1488x over previous
"""Trainium2 Bass kernel for nn_Classifier_42588895707508.

Computation (see reference):
    pool_k[b, h] = max_{s < eff_k[b]} x_k[b, s, h]      (k = 1, 2)
    out[b, c]    = sum_h pool_1[b,h] W[c,h] + pool_2[b,h] W[c, 768+h] + bias[c]
where eff_k[b] is derived from the mask m_k (index of first zero; 0 -> S).

Strategy (memory-bound, ragged sequences):
  * Only the valid prefix of each sample touches the device (~50% of the
    input), packed as fp16 (2e-2 output tolerance; fp16 pooling+matmul
    lands at ~3e-4), halving HBM traffic vs fp32.
  * Host packs, per core, transposed valid blocks x_k[b, :eff, :].T
    (h on partitions, s on the free dim). Rows (= (kind, sample) pairs)
    are distributed round-robin by length rank across the 8 cores, so
    every core gets an identical width structure -> one SPMD program,
    balanced load.
  * DVE reduce_max (InstTensorReduce) has NO accelerated perf modes
    (1 elem/lane/cycle), but tensor_tensor max supports 2x_1p (2 elem/
    lane/cycle for 16-bit). So each DMA tile is laid out as F=8 "planes":
    seq positions of every slot are split across the planes, and the
    planes are folded pairwise with 7 full-tile tensor_max ops (2x rate)
    into a ping-pong accumulator; only the remaining 1/8 of the columns
    go through the slow tensor_reduce. DVE cost: ~0.56 cycles/elem vs
    1.0, dropping it safely below the DMA roofline.
  * Slots are grouped into equal-width reduce groups (widths rounded to
    multiples of F) so one reduce instruction handles g slots x 6 chunks.
    Narrow many-instruction groups are streamed first (overhead hides
    under the DMA stream); wide groups last, with small final tiles so
    the post-DMA tail is short.
  * The tiny linear layer runs on the tensor engine (fp16 matmul, K=128
    per chunk accumulated in fp32 PSUM); per-row partials are summed on
    the host (x1/x2 rows of one sample may land on different cores).
"""

import numpy as np

B, S, H, C = 512, 256, 768, 2
NCORES = 8
CH = H // 128  # 6 h-chunks of 128 partitions
KINDS = 2
SLOTS = B // NCORES  # 64 slots per kind per core
NEG = np.float16(-65504.0)

F = 8  # fold planes per tile
PAD_COST = 1.0  # DP cost of one padded slot-column
INSTR_COST = 20.0  # DP cost of one extra reduce instruction
GROUP_CAP = 6144  # max real columns of one group
TILE_CAP = 24576  # max real columns of one DMA tile
RAMP_UP = [4096, 12288]
TAIL_CAP = 4096  # tile cap near the end of the stream (short DVE tail)
TAIL_ZONE = 8192  # switch to TAIL_CAP when this many columns remain
DATA_BUFS = 3
ACCW_MAX = TILE_CAP // F


def _eff_lengths(m):
    am = np.argmin(np.asarray(m), axis=1)
    return np.where(am == 0, S, am).astype(np.int64)


def _plan_groups(widths):
    """Partition the (descending) width list into contiguous groups.

    Group width is the max member width rounded up to a multiple of F.
    Returns list of (start, n, gw) minimizing PAD_COST * padding +
    INSTR_COST per group, via O(n^2) DP. A group must fit in GROUP_CAP.
    """
    n = len(widths)
    best = np.full(n + 1, np.inf)
    best[0] = 0.0
    prev = np.zeros(n + 1, dtype=np.int64)
    for i in range(1, n + 1):
        for j in range(i - 1, -1, -1):
            w = -(-int(widths[j]) // F) * F  # ceil to multiple of F
            if (i - j) * 6 * w > GROUP_CAP:
                break
            pad = PAD_COST * ((i - j) * w - widths[j:i].sum())
            cost = best[j] + pad + INSTR_COST
            if cost < best[i]:
                best[i] = cost
                prev[i] = j
    groups = []
    i = n
    while i > 0:
        j = prev[i]
        groups.append((j, i - j, -(-int(widths[j]) // F) * F))
        i = j
    groups.reverse()
    return groups


def _build_program(tiles):
    """Build the SPMD Bass program.

    tiles: list of (c0, acc_w, [(o, out_slot, n, v), ...]) in real DRAM
    columns; each tile spans 8*acc_w columns = 8 planes of acc_w; group
    plane-offsets o are tile-local.
    """
    import concourse.bacc as bacc
    import concourse.mybir as mybir
    from concourse.tile import TileContext

    R = sum(F * acc_w for _, acc_w, _ in tiles)
    nc = bacc.Bacc("TRN2", target_bir_lowering=False, debug=False, num_devices=NCORES)
    p_in = nc.dram_tensor("p", [128, R], mybir.dt.float16, kind="ExternalInput")
    wt_in = nc.dram_tensor(
        "wt", [128, KINDS * CH, C], mybir.dt.float16, kind="ExternalInput"
    )
    out_d = nc.dram_tensor(
        "out", [C, 2 * SLOTS], mybir.dt.float32, kind="ExternalOutput"
    )

    with TileContext(nc) as tc:
        with (
            tc.tile_pool(name="data", bufs=DATA_BUFS) as data_pool,
            tc.tile_pool(name="work", bufs=1) as work_pool,
            tc.tile_pool(name="psum", bufs=1, space="PSUM") as psum_pool,
        ):
            wt_t = work_pool.tile([128, KINDS * CH, C], mybir.dt.float16, tag="wt")
            acc = work_pool.tile([128, 2, ACCW_MAX], mybir.dt.float16, tag="acc")
            # pooled[p, slot, ch]: slot = kind*64 + i, partition p = h in chunk
            pooled = work_pool.tile(
                [128, KINDS * SLOTS, CH], mybir.dt.float16, tag="pooled"
            )

            first = True
            for c0, acc_w, tgroups in tiles:
                tw = F * acc_w
                dt = data_pool.tile([128, TILE_CAP], mybir.dt.float16, tag="data")
                # plane-pair sub-DMAs: fold j can start as soon as its pair
                # lands instead of waiting for the whole tile
                for s in range(0, F, 2):
                    lo, hi = s * acc_w, (s + 2) * acc_w
                    nc.sync.dma_start(
                        out=dt[:, lo:hi], in_=p_in[:, c0 + lo : c0 + hi]
                    )
                    if first:
                        # tiny weight load on the ACT HWDGE queue so the
                        # sync queue carries nothing but the data stream
                        nc.scalar.dma_start(out=wt_t, in_=wt_in[:, :, :])
                        first = False
                prev = dt[:, 0:acc_w]
                for j in range(1, F):
                    cur = acc[:, j % 2, :acc_w]
                    nc.vector.tensor_max(
                        out=cur, in0=prev, in1=dt[:, j * acc_w : (j + 1) * acc_w]
                    )
                    prev = cur
                for o, out_slot, gn, v in tgroups:
                    view = prev[:, o : o + gn * 6 * v].rearrange(
                        "p (g v) -> p g v", v=v
                    )
                    nc.vector.reduce_max(
                        out=pooled[:, out_slot : out_slot + gn, :],
                        in_=view,
                        axis=mybir.AxisListType.X,
                    )

            out_sb = work_pool.tile([C, 2 * SLOTS], mybir.dt.float32, tag="osb")
            for k in range(KINDS):
                ps = psum_pool.tile([C, SLOTS], mybir.dt.float32, tag=f"ps{k}")
                for ch in range(CH):
                    nc.tensor.matmul(
                        ps,
                        lhsT=wt_t[:, k * CH + ch, :],
                        rhs=pooled[:, k * SLOTS : (k + 1) * SLOTS, ch],
                        start=(ch == 0),
                        stop=(ch == CH - 1),
                    )
                nc.scalar.copy(out=out_sb[:, k * SLOTS : (k + 1) * SLOTS], in_=ps)
            nc.scalar.dma_start(out=out_d[:, :], in_=out_sb)

    nc.compile()
    return nc


def _plan(effs):
    """Shared planning: groups -> emission order -> tiles -> slot map.

    Returns (tiles, slotinfo, R) where slotinfo[k][i] =
    (c0, acc_w, po, v, w) for packing and tiles drive the program.
    """
    orders = [np.argsort(-effs[k], kind="stable") for k in range(KINDS)]
    slot_w = [effs[k][orders[k][:: NCORES]].astype(np.int64) for k in range(KINDS)]

    raw_groups = []  # (kind, start, n, w)
    for k in range(KINDS):
        for start, n, gw in _plan_groups(slot_w[k]):
            raw_groups.append((k, start, n, gw))
    # narrowest groups first: many small reduces hide under the DMA
    # stream; wide efficient groups last keep the post-DMA tail short
    raw_groups.sort(key=lambda g: g[3])

    total = sum(n * 6 * w for _, _, n, w in raw_groups)

    # assemble tiles of whole groups
    tiles = []  # (c0, acc_w, [(o, out_slot, n, v)])
    slotinfo = [[None] * SLOTS for _ in range(KINDS)]
    col = 0  # real columns consumed (tile bases)
    cur = None  # [c0, pw_sum, [(o, out_slot, n, v)], members]
    consumed = 0

    def close(cur):
        tiles.append((cur[0], cur[1], cur[2]))

    for k, start, n, w in raw_groups:
        v = w // F
        pw = n * 6 * v
        rw = F * pw
        remaining = total - consumed
        cap = (
            RAMP_UP[len(tiles)]
            if len(tiles) < len(RAMP_UP)
            else (TAIL_CAP if remaining <= TAIL_ZONE else TILE_CAP)
        )
        if cur is not None and cur[1] > 0 and F * cur[1] + rw > cap:
            # adding this group would exceed the cap -> close tile
            # (a lone oversized group still gets its own tile)
            close(cur)
            col += F * cur[1]
            cur = None
        if cur is None:
            cur = [col, 0, []]
        o = cur[1]
        cur[2].append((o, k * SLOTS + start, n, v))
        for j in range(n):
            slotinfo[k][start + j] = (cur[0], None, o + j * 6 * v, v, w)
        cur[1] += pw
        consumed += rw
    if cur is not None:
        close(cur)
        col += F * cur[1]
    R = col

    # fix up acc_w in slotinfo (known only at tile close)
    accw_by_c0 = {c0: acc_w for c0, acc_w, _ in tiles}
    for k in range(KINDS):
        for i in range(SLOTS):
            c0, _, po, v, w = slotinfo[k][i]
            slotinfo[k][i] = (c0, accw_by_c0[c0], po, v, w)

    assert all(F * acc_w <= TILE_CAP for _, acc_w, _ in tiles)
    assert R == total
    return tiles, slotinfo, orders, R


_NC_CACHE = {}


def kernel(x1, x2, m1, m2, W, b, _run_opts=None):
    from concourse.bass_utils import run_bass_kernel_spmd

    x1 = np.asarray(x1)
    x2 = np.asarray(x2)
    W = np.asarray(W, dtype=np.float32)
    b = np.asarray(b, dtype=np.float32)
    effs = [_eff_lengths(m1), _eff_lengths(m2)]
    tiles, slotinfo, orders, R = _plan(effs)

    # pack per-core data in fp16, fold-plane layout
    x16 = [np.asarray(x, dtype=np.float32).astype(np.float16) for x in (x1, x2)]
    packs = np.full((NCORES, 128, R), NEG, dtype=np.float16)
    for k in range(KINDS):
        xk, eff, order = x16[k], effs[k], orders[k]
        for i in range(SLOTS):
            c0, acc_w, po, v, w = slotinfo[k][i]
            for c in range(NCORES):
                bidx = order[i * NCORES + c]
                e = int(eff[bidx])
                dst = (
                    packs[c][:, c0 : c0 + F * acc_w]
                    .reshape(128, F, acc_w)[:, :, po : po + 6 * v]
                    .reshape(128, F, 6, v)
                )
                src = xk[bidx, :e, :].T.reshape(6, 128, e).transpose(1, 0, 2)
                jf, rem = e // v, e % v
                if jf:
                    dst[:, :jf] = (
                        src[:, :, : jf * v]
                        .reshape(128, 6, jf, v)
                        .transpose(0, 2, 1, 3)
                    )
                if rem:
                    dst[:, jf, :, :rem] = src[:, :, jf * v :]

    # weights, laid out so lhsT slices are [128 (h), C] per (kind, chunk)
    wtp = (
        np.ascontiguousarray(W.reshape(C, KINDS, CH, 128).transpose(3, 1, 2, 0))
        .reshape(128, KINDS * CH, C)
        .astype(np.float16)
    )

    key = tuple((c0, acc_w, tuple(tg)) for c0, acc_w, tg in tiles)
    nc = _NC_CACHE.get(key)
    if nc is None:
        nc = _build_program(tiles)
        _NC_CACHE[key] = nc
    in_maps = [{"p": packs[c], "wt": wtp} for c in range(NCORES)]

    res = None
    last_err = None
    for _attempt in range(3):
        try:
            res = run_bass_kernel_spmd(
                nc, in_maps, core_ids=list(range(NCORES)), **(_run_opts or {})
            )
            break
        except Exception as e:  # wedged device etc. -- retry
            last_err = e
    if res is None:
        raise last_err

    # combine per-row partials
    out_full = np.zeros((B, C), dtype=np.float32)
    res_all = np.stack([res.results[c]["out"] for c in range(NCORES)])  # [8, C, 128]
    for k in range(KINDS):
        part = res_all[:, :, k * SLOTS : (k + 1) * SLOTS]  # [core, C, slot]
        part = part.transpose(2, 0, 1).reshape(B, C)  # [(slot, core), C]
        out_full[orders[k]] += part
    out_full += b[None, :]
    if _run_opts is not None:
        kernel._last_res = res
    return out_full
